# revision 24
# baseline (speedup 1.0000x reference)
"""GATv2 2-layer + global-mean-pool classifier on 8 Trainium2 NeuronCores.

Strategy (1D node partitioning, dst-sharded edges):
  - 50000 nodes sharded contiguously across 8 cores (6250 each, padded to 6272).
  - Within each core, nodes are sorted by in-degree and grouped into 49
    supertiles of 128 nodes; each node's incident edges are padded to the
    supertile max degree D_t.  Layout: node-on-partition, degree slots along
    the free dimension.
  - Per supertile: one batched indirect DMA per degree slot gathers xl~[src]
    rows (528B/272B) for all 128 nodes from a table in DRAM.
  - e = att.LeakyReLU(xl_j+xr_i) via the identity
        e = (0.6-scaled attl_j+attr_i cols) + sum_c 0.4|att_c|*|xl_c + xr_c|
    with columns sign-permuted so positive / negative blocks are contiguous
    (abs folded into tensor_reduce; the 0.4|att| scale applied on device in
    fp32 so tables stay unscaled).
  - Pad slots point at a poison table row whose attl cols are overwritten to
    -1e4 on device => exp underflows to exactly 0 (no mask tensor shipped);
    all-pad rows are saved from 0/0 by a denominator clamp.
  - Softmax division is deferred past the segment sum; the weighted sum is
    D_t PSUM-accumulated identity-lhsT matmuls.
  - The layer tables are computed shard-locally and AllGather'd.
  - Transfers over the axon tunnel are the wall-clock bottleneck: x ships as
    fp16 in transposed layout, edge indices as uint16, small consts packed
    into one row replicated on device via a rank-1 matmul.  All device_puts
    are issued asynchronously as soon as each host array is ready, and the
    jax/shard_map executable is built once and cached across calls.
"""

import sys

import ml_dtypes
import numpy as np

sys.path.insert(0, "/opt/trn_rl_repo")

F8 = ml_dtypes.float8_e4m3      # what mybir.dt.float8e4 maps to

# ---------------------------------------------------------------- constants
N = 50000
E = 600000
F_IN = 128
HID = 64
NC_CLS = 10
NG = 64
NCORES = 8
NSH_R = N // NCORES          # 6250 real nodes per core
NT = (NSH_R + 127) // 128    # 49 supertiles
NSH = NT * 128               # 6272 padded rank slots per core
TBL_N = NCORES * NSH         # 50176 table rows
POISON = NSH_R               # local rank of the poison row (first pad rank)
F1 = 132                     # L1 table row: 128 feats | attl(2) | pad(2)
F2 = 68                      # L2 table row: 64 feats | attl2(1) | pad(3)
CP = F1 + 128 + F2 + 64      # cpack row: attsc1 | b1 | attsc2 | b2


# ---------------------------------------------------------------- host prep
def _prep_weights(inputs):
    att1 = np.asarray(inputs["att1"], np.float32)
    att2 = np.asarray(inputs["att2"], np.float32)
    Wl1 = np.asarray(inputs["Wl1"], np.float32)
    Wr1 = np.asarray(inputs["Wr1"], np.float32)
    Wl2 = np.asarray(inputs["Wl2"], np.float32)
    Wr2 = np.asarray(inputs["Wr2"], np.float32)
    b1 = np.asarray(inputs["b1"], np.float32)
    b2 = np.asarray(inputs["b2"], np.float32)

    P1 = np.zeros(2 * HID, np.int64)
    k1 = [0, 0]
    Wl1p = np.zeros((F_IN, 2 * HID), np.float32)
    Wr1p = np.zeros((F_IN, 2 * HID), np.float32)
    attsc1 = np.zeros(F1, np.float32)
    for h in (0, 1):
        a = att1[h]
        perm = np.concatenate([np.where(a >= 0)[0], np.where(a < 0)[0]])
        k1[h] = int((a >= 0).sum())
        blk = slice(h * HID, (h + 1) * HID)
        P1[blk] = h * HID + perm
        Wl1p[:, blk] = Wl1[:, blk][:, perm]
        Wr1p[:, blk] = Wr1[:, blk][:, perm]
        attsc1[h * HID:(h + 1) * HID] = 0.4 * np.abs(a[perm])
    wattl1 = 0.6 * np.stack([Wl1[:, h * HID:(h + 1) * HID] @ att1[h]
                             for h in (0, 1)], 1)
    wattr1 = 0.6 * np.stack([Wr1[:, h * HID:(h + 1) * HID] @ att1[h]
                             for h in (0, 1)], 1)
    z2 = np.zeros((F_IN, 2), np.float32)
    Wlp1 = np.concatenate([Wl1p, wattl1, z2], 1).astype(np.float16)
    Wrp1 = np.concatenate([Wr1p, wattr1, z2], 1).astype(np.float16)

    Wl2d = Wl2[P1, :]
    Wr2d = Wr2[P1, :]
    a2 = att2[0]
    P2 = np.concatenate([np.where(a2 >= 0)[0], np.where(a2 < 0)[0]])
    k2 = int((a2 >= 0).sum())
    attsc2 = np.zeros(F2, np.float32)
    attsc2[:HID] = 0.4 * np.abs(a2[P2])
    wattl2 = 0.6 * (Wl2d @ a2)[:, None]
    wattr2 = 0.6 * (Wr2d @ a2)[:, None]
    z3 = np.zeros((2 * HID, 3), np.float32)
    Wlp2 = np.concatenate([Wl2d[:, P2], wattl2, z3], 1).astype(np.float32)
    Wrp2 = np.concatenate([Wr2d[:, P2], wattr2, z3], 1).astype(np.float32)

    cpack = np.concatenate([attsc1, b1[P1], attsc2, b2[P2]]).astype(np.float32)
    return dict(Wlp1=Wlp1, Wrp1=Wrp1, Wlp2=Wlp2, Wrp2=Wrp2, cpack=cpack,
                P2=P2, k1=k1, k2=k2)


def _prep_graph(ei):
    """Degree-sort node partition + supertile degree profile."""
    src = np.concatenate([ei[0].astype(np.int32),
                          np.arange(N, dtype=np.int32)])
    dst = np.concatenate([ei[1].astype(np.int32),
                          np.arange(N, dtype=np.int32)])
    deg = np.bincount(dst, minlength=N).astype(np.int32)
    assert deg.max() <= 128, f"max degree {deg.max()} > 128"
    deg2 = deg.reshape(NCORES, NSH_R)
    order = np.argsort(-deg2, axis=1, kind="stable")
    degs = np.take_along_axis(deg2, order, axis=1)
    degsp = np.zeros((NCORES, NSH), np.int32)
    degsp[:, :NSH_R] = degs
    D = np.maximum(degsp[:, ::128].max(axis=0), 1)
    off = np.concatenate([[0], np.cumsum(D)]).astype(np.int64)
    perm_nodes = order + (np.arange(NCORES, dtype=np.int32) * NSH_R)[:, None]
    rank_of = np.empty(N, np.int32)
    rank_of[perm_nodes.ravel()] = np.tile(
        np.arange(NSH_R, dtype=np.int32), NCORES)
    return src, dst, D, off, perm_nodes, rank_of


XQ_SCALE = 32.0      # u8 linear quant step for x: q = rint(x*32)+128


def _quant_x(x, cols):
    """u8-quantize a column block of x, natural node order (the device
    permutation-gathers rows per supertile)."""
    return np.clip(np.rint(x[:, cols] * XQ_SCALE) + 128.0, 0, 255
                   ).astype(np.uint8)


def _build_perm(perm_nodes):
    """Local node id of each (partition, supertile) slot, u16."""
    op = np.zeros((NCORES, NSH), np.uint16)
    op[:, :NSH_R] = perm_nodes - (np.arange(NCORES) * NSH_R)[:, None]
    return np.ascontiguousarray(
        op.reshape(NCORES, NT, 128).transpose(0, 2, 1)).reshape(-1, NT)


def _build_edges(src, dst, D, off, rank_of, SD):
    gkey = ((dst // NSH_R) * NSH + rank_of[dst]).astype(np.uint16)
    eorder = np.argsort(gkey, kind="stable")   # 2-pass radix on u16
    gs = gkey[eorder].astype(np.int32)
    vals = ((src // NSH_R) * NSH + rank_of[src]).astype(np.uint16)[eorder]
    starts = np.searchsorted(gs, np.arange(TBL_N + 1, dtype=np.int32)
                             ).astype(np.int64)
    slot = np.arange(len(gs), dtype=np.int64) - starts[gs]
    c_e = gs // NSH
    r_e = gs % NSH
    idx_cat = np.full((NCORES * 128, SD), POISON, np.uint16)
    idx_cat[c_e * 128 + (r_e & 127), off[r_e >> 7] + slot] = vals
    return idx_cat


def _build_batch(batch_np, perm_nodes):
    bpad = np.full((NCORES, NSH), -1.0, np.float32)
    bpad[:, :NSH_R] = batch_np[perm_nodes].astype(np.float32)
    return np.ascontiguousarray(
        bpad.reshape(NCORES, NT, 128).transpose(0, 2, 1)
    ).reshape(NCORES * 128, NT)


def prep(inputs):
    """Full host-side restructuring (single-shot path used by the mock)."""
    w = _prep_weights(inputs)
    ei = np.asarray(inputs["edge_index"])
    src, dst, D, off, perm_nodes, rank_of = _prep_graph(ei)
    SD = int(D.sum())
    static = dict(D=[int(d) for d in D], SD=SD, k1=w["k1"], k2=w["k2"])
    x = np.asarray(inputs["x"], np.float32)
    arrs = {
        "xq0": _quant_x(x, slice(0, 64)),
        "xq1": _quant_x(x, slice(64, 128)),
        "permv": _build_perm(perm_nodes),
        "wg16": np.tile(np.concatenate([w["Wlp1"], w["Wrp1"]], 1)[None],
                        (NCORES, 1, 1)).reshape(-1, 2 * F1),
        "wg32": np.concatenate([
            np.tile(np.concatenate([w["Wlp2"], w["Wrp2"]], 1)[None],
                    (NCORES, 1, 1)).reshape(-1, 2 * F2),
            _build_batch(np.asarray(inputs["batch"]).astype(np.int32),
                         perm_nodes)], 1),
        "idxu": _build_edges(src, dst, D, off, rank_of, SD),
        "cpack": np.tile(w["cpack"][None], (NCORES, 1)),
    }
    host_ctx = dict(
        batch=np.asarray(inputs["batch"]).astype(np.int32), P2=w["P2"],
        Wlin=np.asarray(inputs["Wlin"], np.float32),
        blin=np.asarray(inputs["blin"], np.float32),
    )
    return static, arrs, host_ctx


def host_epilogue(pooled, host_ctx):
    """pooled: [NG, HID] already summed across cores (device AllReduce)."""
    counts = np.bincount(host_ctx["batch"], minlength=NG).astype(np.float32)
    g = pooled / np.maximum(counts, 1.0)[:, None]
    Wlin_p = host_ctx["Wlin"][host_ctx["P2"], :]
    return (g @ Wlin_p + host_ctx["blin"]).astype(np.float32)


# ---------------------------------------------------------------- numpy mock
def numpy_device_mock(static, arrs, host_ctx):
    """Bit-faithful-ish (fp32 with fp16-rounded inputs) simulation of the
    device kernel.  Used to validate host-side restructuring off-hardware."""
    D, SD = static["D"], static["SD"]
    off = np.concatenate([[0], np.cumsum(D)]).astype(np.int64)
    k1, k2 = static["k1"], static["k2"]
    xq = np.concatenate([arrs["xq0"], arrs["xq1"]], 1).astype(np.float32)
    x16 = ((xq - 128.0) * np.float32(1.0 / XQ_SCALE)).astype(np.float16)
    permv = arrs["permv"].reshape(NCORES, 128, NT).astype(np.int64)
    xg = np.zeros((NCORES, NSH, F_IN), np.float16)
    for c in range(NCORES):
        loc = np.ascontiguousarray(
            permv[c].T).reshape(-1)               # rank -> local node id
        xg[c] = x16.reshape(NCORES, NSH_R, F_IN)[c][loc]
    xT = np.ascontiguousarray(xg.transpose(0, 2, 1)).astype(np.float32)
    wlp1 = arrs["wg16"][:F_IN, 0:F1].astype(np.float32)
    wrp1 = arrs["wg16"][:F_IN, F1:2 * F1].astype(np.float32)
    wlp2 = arrs["wg32"][:2 * HID, 0:F2]
    wrp2 = arrs["wg32"][:2 * HID, F2:2 * F2]
    idx = arrs["idxu"].reshape(NCORES, 128, SD).astype(np.int64)
    cpk = arrs["cpack"][0]
    attsc1 = cpk[0:F1]
    b1r = cpk[F1:F1 + 128]
    attsc2 = cpk[F1 + 128:F1 + 128 + F2]
    b2r = cpk[F1 + 128 + F2:CP]
    batchv = arrs["wg32"][:, 2 * F2:2 * F2 + NT].reshape(NCORES, 128, NT)

    def edge_layer(tbl, xre, Fw, nheads, kpos, attsc, brow, h_w):
        h_all = np.zeros((NCORES, 128, NT * h_w), np.float32)
        for c in range(NCORES):
            for t in range(NT):
                d = D[t]
                A = tbl[idx[c, :, off[t]:off[t] + d].reshape(-1)].reshape(
                    128, d, Fw)
                xr = xre[c, :, t * Fw:(t + 1) * Fw]
                s = (A + xr[:, None, :]) * attsc[None, None, :]
                e = np.zeros((128, nheads, d), np.float32)
                for h in range(nheads):
                    base = h * HID
                    pos = np.abs(s[:, :, base:base + kpos[h]]).sum(2)
                    neg = np.abs(s[:, :, base + kpos[h]:base + HID]).sum(2)
                    attl = A[:, :, h_w + h] if Fw == F1 else A[:, :, HID + h]
                    attr = xr[:, (128 if Fw == F1 else HID) + h]
                    e[:, h] = (attl + attr[:, None]) + (pos - neg)
                p = np.exp(e)
                den = np.maximum(p.sum(2), 1e-30)
                outw = np.zeros((128, h_w), np.float32)
                for h in range(nheads):
                    outw[:, h * HID:(h + 1) * HID] = (
                        A[:, :, h * HID:(h + 1) * HID]
                        * p[:, h, :, None]).sum(1) / den[:, h:h + 1]
                hh = outw + brow[None, :h_w]
                hh = np.maximum(hh, np.exp(np.minimum(hh, 0.0)) - 1.0)
                h_all[c, :, t * h_w:(t + 1) * h_w] = hh
        return h_all

    tbl1 = np.zeros((TBL_N, F1), np.float32)
    xre1 = np.zeros((NCORES, 128, NT * F1), np.float32)
    for c in range(NCORES):
        for t in range(NT):
            xsl = xT[c][:, t * 128:(t + 1) * 128]
            tbl1[c * NSH + t * 128:c * NSH + (t + 1) * 128] = xsl.T @ wlp1
            xre1[c, :, t * F1:(t + 1) * F1] = xsl.T @ wrp1
    tbl1[np.arange(NCORES) * NSH + POISON, 128:130] = -1e4
    h1 = edge_layer(tbl1, xre1, F1, 2, k1, attsc1, cpk[F1:F1 + 128], 128)

    tbl2 = np.zeros((TBL_N, F2), np.float32)
    xre2 = np.zeros((NCORES, 128, NT * F2), np.float32)
    for c in range(NCORES):
        for t in range(NT):
            h1t = h1[c, :, t * 128:(t + 1) * 128]
            tbl2[c * NSH + t * 128:c * NSH + (t + 1) * 128] = h1t @ wlp2
            xre2[c, :, t * F2:(t + 1) * F2] = h1t @ wrp2
    tbl2[np.arange(NCORES) * NSH + POISON, HID:HID + 1] = -1e4
    h2 = edge_layer(tbl2, xre2, F2, 1, [k2], attsc2, b2r, HID)

    pooled = np.zeros((NCORES, NG, HID), np.float32)
    for c in range(NCORES):
        for t in range(NT):
            onehot = (np.arange(NG, dtype=np.float32)[None, :]
                      == batchv[c, :, t:t + 1]).astype(np.float32)
            pooled[c] += onehot.T @ h2[c, :, t * HID:(t + 1) * HID]
    return host_epilogue(pooled.sum(0), host_ctx)


# ---------------------------------------------------------------- device impl
def build_nc(static):
    import concourse.bass as bass
    import concourse.bacc as bacc
    import concourse.mybir as mybir
    import concourse.tile as tile
    from contextlib import ExitStack

    fp32 = mybir.dt.float32
    fp16 = mybir.dt.float16
    i32 = mybir.dt.int32
    u16 = mybir.dt.uint16
    u8 = mybir.dt.uint8
    AF = mybir.ActivationFunctionType
    OP = mybir.AluOpType

    D, SD = static["D"], static["SD"]
    off = np.concatenate([[0], np.cumsum(D)]).astype(np.int64)
    k1, k2 = static["k1"], static["k2"]

    nc = bacc.Bacc(None, num_devices=NCORES)

    # ---- I/O ----
    xq0 = nc.dram_tensor("xq0", [NSH_R, 64], u8, kind="ExternalInput")
    xq1 = nc.dram_tensor("xq1", [NSH_R, 64], u8, kind="ExternalInput")
    permv = nc.dram_tensor("permv", [128, NT], u16, kind="ExternalInput")
    wg16 = nc.dram_tensor("wg16", [F_IN, 2 * F1], fp16, kind="ExternalInput")
    wg32 = nc.dram_tensor("wg32", [128, 2 * F2 + NT], fp32,
                          kind="ExternalInput")
    idxu = nc.dram_tensor("idxu", [128, SD], u16, kind="ExternalInput")
    cpack = nc.dram_tensor("cpack", [1, CP], fp32, kind="ExternalInput")
    pooled_out = nc.dram_tensor("pooled", [NG, HID], fp32,
                                kind="ExternalOutput")

    # collective buffers (internal DRAM)
    tbl1_sh = nc.dram_tensor("tbl1_sh", [NSH, F1], fp32)
    tbl1 = nc.dram_tensor("tbl1", [TBL_N, F1], fp32, addr_space="Shared")
    tbl2_sh = nc.dram_tensor("tbl2_sh", [NSH, F2], fp32)
    tbl2 = nc.dram_tensor("tbl2", [TBL_N, F2], fp32, addr_space="Shared")
    pool_loc = nc.dram_tensor("pool_loc", [NG, HID], fp32)
    pool_red = nc.dram_tensor("pool_red", [NG, HID], fp32)

    with tile.TileContext(nc) as tc, ExitStack() as ctx:
        cp = ctx.enter_context(tc.tile_pool(name="const", bufs=1))
        wg16_s = cp.tile([F_IN, 2 * F1], fp16)
        nc.sync.dma_start(wg16_s[:], wg16[:, :])
        wlp1_s = wg16_s[:, 0:F1]
        wrp1_s = wg16_s[:, F1:2 * F1]
        wg32_s = cp.tile([128, 2 * F2 + NT], fp32)
        nc.sync.dma_start(wg32_s[:], wg32[:, :])
        wlp2_s = wg32_s[:, 0:F2]
        wrp2_s = wg32_s[:, F2:2 * F2]
        batch_s = wg32_s[:, 2 * F2:2 * F2 + NT]
        cpk_s = cp.tile([1, CP], fp32); nc.sync.dma_start(cpk_s[:], cpack[:, :])
        idxu_s = cp.tile([128, SD], u16); nc.sync.dma_start(idxu_s[:], idxu[:, :])
        idx32_s = cp.tile([128, SD], i32)
        nc.vector.tensor_scalar(idx32_s[:], idxu_s[:], 0, None, op0=OP.add)
        permu_s = cp.tile([128, NT], u16); nc.sync.dma_start(permu_s[:], permv[:, :])
        perm32_s = cp.tile([128, NT], i32)
        nc.vector.tensor_scalar(perm32_s[:], permu_s[:], 0, None, op0=OP.add)

        ones_s = cp.tile([1, 128], fp32); nc.vector.memset(ones_s[:], 1.0)
        pois_s = cp.tile([1, 2], fp32); nc.vector.memset(pois_s[:], -1e4)
        iotaF_i = cp.tile([128, 128], i32)
        nc.gpsimd.iota(iotaF_i[:], [[1, 128]], channel_multiplier=0)
        iotaP_i = cp.tile([128, 1], i32)
        nc.gpsimd.iota(iotaP_i[:], [[1, 1]], channel_multiplier=1)
        iotaF_f = cp.tile([128, 128], fp32)
        nc.vector.tensor_scalar(iotaF_f[:], iotaF_i[:], 0, None, op0=OP.add)
        iotaP_f = cp.tile([128, 1], fp32)
        nc.vector.tensor_scalar(iotaP_f[:], iotaP_i[:], 0, None, op0=OP.add)
        id_s = cp.tile([128, 128], fp32)
        nc.vector.tensor_scalar(id_s[:], iotaF_f[:], iotaP_f[:, 0:1], None,
                                op0=OP.is_equal)
        id16_s = cp.tile([128, 128], fp16)
        nc.vector.tensor_scalar(id16_s[:], id_s[:], 0.0, None, op0=OP.add)
        io64_s = iotaF_f[:, 0:NG]

        consts_s = cp.tile([128, CP], fp32)
        with tc.tile_pool(name="init_ps", bufs=1, space="PSUM") as ip:
            psC = ip.tile([128, CP], fp32)
            nc.tensor.matmul(psC[:], ones_s[:], cpk_s[:], start=True, stop=True)
            nc.scalar.copy(consts_s[:], psC[:])
        attsc1_s = consts_s[:, 0:F1]
        b1_s = consts_s[:, F1:F1 + 128]
        attsc2_s = consts_s[:, F1 + 128:F1 + 128 + F2]
        b2_s = consts_s[:, F1 + 128 + F2:CP]

        big = ctx.enter_context(tc.tile_pool(name="big", bufs=1))
        xre1_s = big.tile([128, NT * F1], fp32)
        h1_s = big.tile([128, NT * 128], fp32)

        # ---------------- phase A: layer-1 tables ----------------
        # x arrives [nodes, feat] u8-quantized in natural node order; per
        # supertile: permutation-gather 128 rows, dequantize to fp16,
        # PE-transpose, then the two table matmuls
        with tc.tile_pool(name="phA", bufs=3) as pa, \
             tc.tile_pool(name="phA_ps", bufs=3, space="PSUM") as pap:
            for t in range(NT):
                xqt = pa.tile([128, F_IN], u8, tag="xqt")
                for j, xq in enumerate((xq0, xq1)):
                    nc.gpsimd.indirect_dma_start(
                        out=xqt[:, j * 64:(j + 1) * 64],
                        out_offset=None,
                        in_=xq[:, :],
                        in_offset=bass.IndirectOffsetOnAxis(
                            ap=perm32_s[:, t:t + 1], axis=0),
                    )
                x16t = pa.tile([128, F_IN], fp16, tag="x16t")
                nc.vector.tensor_scalar(x16t[:], xqt[:], -128.0,
                                        float(1.0 / XQ_SCALE),
                                        op0=OP.add, op1=OP.mult)
                psT = pap.tile([128, 128], fp16, tag="psT")
                nc.tensor.transpose(psT[:], x16t[:], id16_s[:])
                lhs = pa.tile([128, 128], fp16, tag="xTt")
                nc.scalar.copy(lhs[:], psT[:])
                ps = pap.tile([128, F1], fp32, tag="psA")
                nc.tensor.matmul(ps[:], lhs[:], wlp1_s, start=True, stop=True)
                stg = pa.tile([128, F1], fp32, tag="stgA")
                nc.scalar.copy(stg[:], ps[:])
                nc.sync.dma_start(tbl1_sh[t * 128:(t + 1) * 128, :], stg[:])
                ps2 = pap.tile([128, F1], fp32, tag="psA")
                nc.tensor.matmul(ps2[:], lhs[:], wrp1_s, start=True, stop=True)
                nc.scalar.copy(xre1_s[:, t * F1:(t + 1) * F1], ps2[:])
        nc.sync.dma_start(tbl1_sh[POISON:POISON + 1, 128:130], pois_s[0:1, 0:2])

        nc.gpsimd.collective_compute(
            "AllGather", mybir.AluOpType.bypass,
            replica_groups=[list(range(NCORES))],
            ins=[tbl1_sh[:, :]], outs=[tbl1[:, :]],
        )

        # ---------------- edge phase ----------------
        def edge_layer(tblT, xre_s, Fw, nheads, kpos, attsc_s, bt_s, h_out,
                       h_w):
            maxD = max(D)
            with tc.tile_pool(name=f"edg{Fw}", bufs=2) as pe, \
                 tc.tile_pool(name=f"sm{Fw}", bufs=3) as psm, \
                 tc.tile_pool(name=f"ps{Fw}", bufs=2, space="PSUM") as pps:
                for t in range(NT):
                    d = D[t]
                    # idx/out for the indirect DMA must be exact contiguous
                    # tiles (sliced/strided APs crash the DMA engine)
                    idxt = pe.tile([128, d], i32, tag="idxt")
                    nc.vector.tensor_scalar(
                        idxt[:], idx32_s[:, int(off[t]):int(off[t]) + d],
                        0, None, op0=OP.add)
                    A = pe.tile([128, d * Fw], fp32, tag="A")
                    # HW indirect DMA honors ONE offset per partition per call
                    for kk in range(d):
                        nc.gpsimd.indirect_dma_start(
                            out=A[:, kk * Fw:(kk + 1) * Fw],
                            out_offset=None,
                            in_=tblT[:, :],
                            in_offset=bass.IndirectOffsetOnAxis(
                                ap=idxt[:, kk:kk + 1], axis=0),
                        )
                    A3 = A[:].rearrange("p (d f) -> p d f", f=Fw)
                    xr = xre_s[:, t * Fw:(t + 1) * Fw]
                    xrb = xr.rearrange("p (o f) -> p o f", o=1).to_broadcast(
                        [128, d, Fw])
                    s = pe.tile([128, maxD * Fw], fp32, tag="s")
                    s3 = s[:, :d * Fw].rearrange("p (d f) -> p d f", f=Fw)
                    nc.vector.tensor_tensor(s3, A3, xrb, op=OP.add)
                    ascb = attsc_s.rearrange("p (o f) -> p o f",
                                             o=1).to_broadcast([128, d, Fw])
                    nc.vector.tensor_tensor(s3, s3, ascb, op=OP.mult)
                    # e-work tile: [pos_h, neg_h] x heads, then e [h, d]
                    ew = psm.tile([128, 4 * maxD], fp32, tag="ew")
                    for h in range(nheads):
                        base = h * HID
                        nc.vector.tensor_reduce(
                            ew[:, (2 * h) * d:(2 * h) * d + d],
                            s3[:, :, base:base + kpos[h]],
                            axis=mybir.AxisListType.X, op=OP.add,
                            apply_absolute_value=True)
                        nc.vector.tensor_reduce(
                            ew[:, (2 * h + 1) * d:(2 * h + 1) * d + d],
                            s3[:, :, base + kpos[h]:base + HID],
                            axis=mybir.AxisListType.X, op=OP.add,
                            apply_absolute_value=True)
                    # pn = pos - neg  -> [128, h, d]
                    pn = psm.tile([128, 2 * maxD], fp32, tag="pn")
                    ew4 = ew[:, :4 * d].rearrange("p (s d) -> p s d", d=d)
                    pnv = pn[:, :nheads * d].rearrange("p (s d) -> p s d", d=d)
                    nc.vector.tensor_tensor(
                        pnv, ew4[:, 0:2 * nheads:2, :],
                        ew4[:, 1:2 * nheads:2, :], op=OP.subtract)
                    # e = (attl + attr) + pn   (0.6 folded into watt cols)
                    ac = 128 if Fw == F1 else HID
                    attr = xr[:, ac:][:, :nheads]
                    attrb = bass.AP(attr.tensor, attr.offset,
                                    [attr.ap[0], [1, nheads], [0, d]])
                    attlv = bass.AP(A.tensor, A.offset + ac,
                                    [A.ap[0], [1, nheads], [Fw, d]])
                    tmp = psm.tile([128, 2 * maxD], fp32, tag="tmp")
                    tmpv = tmp[:, :nheads * d].rearrange("p (s d) -> p s d", d=d)
                    nc.vector.tensor_tensor(tmpv, attlv, attrb, op=OP.add)
                    ee = psm.tile([128, 2 * maxD], fp32, tag="ee")
                    eev = ee[:, :nheads * d].rearrange("p (s d) -> p s d", d=d)
                    nc.vector.tensor_tensor(eev, tmpv, pnv, op=OP.add)
                    # exp
                    pexp = psm.tile([128, 2 * maxD], fp32, tag="pexp")
                    pexpv = pexp[:, :nheads * d]
                    nc.scalar.activation(pexpv, ee[:, :nheads * d], AF.Exp)
                    pexp3 = pexpv.rearrange("p (s d) -> p s d", d=d)
                    # denom + clamp (all-pad rows sum to exactly 0) + recip
                    den = psm.tile([128, 2], fp32, tag="den")
                    nc.vector.tensor_reduce(den[:, :nheads], pexp3,
                                            axis=mybir.AxisListType.X,
                                            op=OP.add)
                    rd = psm.tile([128, 2], fp32, tag="rd")
                    nc.vector.tensor_scalar(rd[:, :nheads], den[:, :nheads],
                                            1e-30, None, op0=OP.max)
                    nc.vector.reciprocal(rd[:, :nheads], rd[:, :nheads])
                    # W = A * exp  (per head)
                    W = pe.tile([128, maxD * h_w], fp32, tag="W")
                    W3 = W[:, :d * h_w].rearrange("p (d f) -> p d f", f=h_w)
                    for h in range(nheads):
                        eb = bass.AP(pexp.tensor, pexp.offset + h * d,
                                     [pexp.ap[0], [1, d], [0, HID]])
                        nc.vector.tensor_tensor(
                            W3[:, :, h * HID:(h + 1) * HID],
                            A3[:, :, h * HID:(h + 1) * HID], eb, op=OP.mult)
                    # PSUM-accumulated identity matmuls over slots
                    po = pps.tile([128, h_w], fp32, tag="po")
                    for dd in range(d):
                        nc.tensor.matmul(po[:], id_s[:], W3[:, dd, :],
                                         start=(dd == 0), stop=(dd == d - 1))
                    # epilogue: divide by denom (ACT copy*scale), bias, elu
                    hh = psm.tile([128, h_w], fp32, tag="hh")
                    for h in range(nheads):
                        nc.scalar.activation(
                            hh[:, h * HID:(h + 1) * HID],
                            po[:, h * HID:(h + 1) * HID],
                            AF.Copy, bias=0.0, scale=rd[:, h:h + 1])
                    nc.vector.tensor_tensor(hh[:], hh[:], bt_s[:, :h_w],
                                            op=OP.add)
                    # elu: max(x, exp(min(x,0)) - 1)
                    mn = psm.tile([128, h_w], fp32, tag="mn")
                    nc.vector.tensor_scalar(mn[:], hh[:], 0.0, None, op0=OP.min)
                    ex = psm.tile([128, h_w], fp32, tag="ex")
                    nc.scalar.activation(ex[:], mn[:], AF.Exp)
                    nc.vector.scalar_tensor_tensor(
                        h_out[:, t * h_w:(t + 1) * h_w], ex[:], -1.0, hh[:],
                        op0=OP.add, op1=OP.max)

        edge_layer(tbl1, xre1_s, F1, 2, k1, attsc1_s, b1_s, h1_s, 128)

        # ---------------- phase C: layer-2 tables ----------------
        xre2_s = big.tile([128, NT * F2], fp32)
        with tc.tile_pool(name="phC", bufs=3) as pc, \
             tc.tile_pool(name="phC_ps", bufs=3, space="PSUM") as pcp:
            for t in range(NT):
                psT = pcp.tile([128, 128], fp32, tag="psT")
                nc.tensor.transpose(psT[:], h1_s[:, t * 128:(t + 1) * 128],
                                    id_s[:])
                h1T = pc.tile([128, 128], fp32, tag="h1T")
                nc.scalar.copy(h1T[:], psT[:])
                ps = pcp.tile([128, F2], fp32, tag="psC")
                nc.tensor.matmul(ps[:], h1T[:], wlp2_s, start=True, stop=True)
                stg = pc.tile([128, F2], fp32, tag="stgC")
                nc.scalar.copy(stg[:], ps[:])
                nc.sync.dma_start(tbl2_sh[t * 128:(t + 1) * 128, :], stg[:])
                ps2 = pcp.tile([128, F2], fp32, tag="psC")
                nc.tensor.matmul(ps2[:], h1T[:], wrp2_s, start=True, stop=True)
                nc.scalar.copy(xre2_s[:, t * F2:(t + 1) * F2], ps2[:])
        nc.sync.dma_start(tbl2_sh[POISON:POISON + 1, HID:HID + 1],
                          pois_s[0:1, 0:1])

        nc.gpsimd.collective_compute(
            "AllGather", mybir.AluOpType.bypass,
            replica_groups=[list(range(NCORES))],
            ins=[tbl2_sh[:, :]], outs=[tbl2[:, :]],
        )

        # ---------------- phase D: layer-2 edges ----------------
        h2_s = big.tile([128, NT * HID], fp32)
        edge_layer(tbl2, xre2_s, F2, 1, [k2], attsc2_s, b2_s, h2_s, HID)

        # ---------------- phase E: pooling + cross-core reduce ----------------
        with tc.tile_pool(name="phE", bufs=3) as pe_, \
             tc.tile_pool(name="phE_ps", bufs=1, space="PSUM") as pep:
            psP = pep.tile([NG, HID], fp32)
            for t in range(NT):
                oh = pe_.tile([128, NG], fp32, tag="oh")
                nc.vector.tensor_scalar(oh[:], io64_s, batch_s[:, t:t + 1],
                                        None, op0=OP.is_equal)
                nc.tensor.matmul(psP[:], oh[:], h2_s[:, t * HID:(t + 1) * HID],
                                 start=(t == 0), stop=(t == NT - 1))
            stg = pe_.tile([NG, HID], fp32, tag="stgE")
            nc.scalar.copy(stg[:], psP[:])
            nc.sync.dma_start(pool_loc[:, :], stg[:])
            # AllReduce -> every core holds the full pool; the host then
            # fetches a single shard (1 RTT instead of 8)
            nc.gpsimd.collective_compute(
                "AllReduce", mybir.AluOpType.add,
                replica_groups=[list(range(NCORES))],
                ins=[pool_loc[:, :]], outs=[pool_red[:, :]],
            )
            rstg = pe_.tile([NG, HID], fp32, tag="rstg")
            nc.sync.dma_start(rstg[:], pool_red[:, :])
            nc.sync.dma_start(pooled_out[:, :], rstg[:])

    nc.finalize()
    return nc


# ---------------------------------------------------------------- runner
class _Runner:
    """Builds the Bass module + shard_map'd jit executable ONCE; later calls
    reuse it (no retracing).  device_put is async -> callers overlap H2D with
    the rest of host prep."""

    def __init__(self, static):
        import jax
        import concourse.mybir as mybir
        from jax.sharding import Mesh, PartitionSpec, NamedSharding
        from jax.experimental.shard_map import shard_map
        from concourse.bass2jax import (
            _bass_exec_p, partition_id_tensor, install_neuronx_cc_hook)

        install_neuronx_cc_hook()
        self.jax = jax
        nc = build_nc(static)
        self.nc = nc
        pname = nc.partition_id_tensor.name if nc.partition_id_tensor else None
        in_names, out_names, out_avals, zero_shapes = [], [], [], []
        for alloc in nc.m.functions[0].allocations:
            if not isinstance(alloc, mybir.MemoryLocationSet):
                continue
            name = alloc.memorylocations[0].name
            if alloc.kind == "ExternalInput":
                if name != pname:
                    in_names.append(name)
            elif alloc.kind == "ExternalOutput":
                shape = tuple(alloc.tensor_shape)
                dtype = mybir.dt.np(alloc.dtype)
                out_names.append(name)
                out_avals.append(jax.core.ShapedArray(shape, dtype))
                zero_shapes.append((shape, dtype))
        self.dbg_name = None
        if nc.dbg_addr is not None:
            assert not nc.dbg_callbacks
            self.dbg_name = nc.dbg_addr.name
            in_names.append(self.dbg_name)
        n_params = len(in_names)
        all_names = in_names + out_names + ([pname] if pname else [])
        self.in_names = in_names
        self.out_names = out_names
        self.zero_shapes = zero_shapes
        donate = tuple(range(n_params, n_params + len(out_names)))

        def _body(*args):
            operands = list(args)
            if pname is not None:
                operands.append(partition_id_tensor())
            return tuple(_bass_exec_p.bind(
                *operands, out_avals=tuple(out_avals),
                in_names=tuple(all_names), out_names=tuple(out_names),
                lowering_input_output_aliases=(),
                sim_require_finite=True, sim_require_nnan=True, nc=nc))

        devices = jax.devices()[:NCORES]
        mesh = Mesh(np.asarray(devices), ("core",))
        self.sharding = NamedSharding(mesh, PartitionSpec("core"))
        nio = n_params + len(out_names)
        self.fn = jax.jit(
            shard_map(_body, mesh=mesh,
                      in_specs=(PartitionSpec("core"),) * nio,
                      out_specs=(PartitionSpec("core"),) * len(out_names),
                      check_rep=False),
            donate_argnums=donate, keep_unused=True)

    def put(self, arr):
        return self.jax.device_put(arr, self.sharding)

    def put_zeros(self):
        return [self.put(np.zeros((NCORES * s[0],) + tuple(s[1:]), dt))
                for s, dt in self.zero_shapes]

    def run(self, handles, zeros):
        if self.dbg_name is not None and self.dbg_name not in handles:
            handles[self.dbg_name] = self.put(
                np.zeros((NCORES, 2), np.uint32))
        outs = self.fn(*[handles[n] for n in self.in_names], *zeros)
        # pooled was AllReduced on device; shard 0 already holds the full sum
        pooled = outs[self.out_names.index("pooled")]
        return np.asarray(pooled.addressable_shards[0].data)


_CACHE = {}


def _get_runner(static):
    key = (tuple(static["D"]), tuple(static["k1"]), static["k2"])
    if key not in _CACHE:
        _CACHE[key] = _Runner(static)
    return _CACHE[key]


def kernel(**inputs) -> np.ndarray:
    w = _prep_weights(inputs)
    ei = np.asarray(inputs["edge_index"])
    src, dst, D, off, perm_nodes, rank_of = _prep_graph(ei)
    SD = int(D.sum())
    static = dict(D=[int(d) for d in D], SD=SD, k1=w["k1"], k2=w["k2"])
    runner = _get_runner(static)

    # x first, in two column chunks: device_put is async, so the first 3MB
    # start streaming while the second chunk is still being quantized and
    # the rest of host prep runs under the transfer
    x = np.asarray(inputs["x"], np.float32)
    handles = {"xq0": runner.put(_quant_x(x, slice(0, 64)))}
    handles["xq1"] = runner.put(_quant_x(x, slice(64, 128)))
    handles["permv"] = runner.put(_build_perm(perm_nodes))
    zeros = runner.put_zeros()
    handles["idxu"] = runner.put(_build_edges(src, dst, D, off, rank_of, SD))
    handles["wg16"] = runner.put(
        np.tile(np.concatenate([w["Wlp1"], w["Wrp1"]], 1)[None],
                (NCORES, 1, 1)).reshape(-1, 2 * F1))
    batch_np = np.asarray(inputs["batch"]).astype(np.int32)
    handles["wg32"] = runner.put(np.concatenate([
        np.tile(np.concatenate([w["Wlp2"], w["Wrp2"]], 1)[None],
                (NCORES, 1, 1)).reshape(-1, 2 * F2),
        _build_batch(batch_np, perm_nodes)], 1))
    handles["cpack"] = runner.put(np.tile(w["cpack"][None], (NCORES, 1)))

    pooled = runner.run(handles, zeros)
    host_ctx = dict(batch=batch_np, P2=w["P2"],
                    Wlin=np.asarray(inputs["Wlin"], np.float32),
                    blin=np.asarray(inputs["blin"], np.float32))
    return host_epilogue(pooled, host_ctx)


# revision 32
# speedup vs baseline: 1.0093x; 1.0093x over previous
"""GATv2 2-layer + global-mean-pool classifier on 8 Trainium2 NeuronCores.

Strategy (1D node partitioning, dst-sharded edges):
  - 50000 nodes sharded contiguously across 8 cores (6250 each, padded to 6272).
  - Within each core, nodes are sorted by in-degree and grouped into 49
    supertiles of 128 nodes; each node's incident edges are padded to the
    supertile max degree D_t.  Layout: node-on-partition, degree slots along
    the free dimension.
  - Per supertile: one batched indirect DMA per degree slot gathers xl~[src]
    rows (528B/272B) for all 128 nodes from a table in DRAM.
  - e = att.LeakyReLU(xl_j+xr_i) via the identity
        e = (0.6-scaled attl_j+attr_i cols) + sum_c 0.4|att_c|*|xl_c + xr_c|
    with columns sign-permuted so positive / negative blocks are contiguous
    (abs folded into tensor_reduce; the 0.4|att| scale applied on device in
    fp32 so tables stay unscaled).
  - Pad slots point at a poison table row whose attl cols are overwritten to
    -1e4 on device => exp underflows to exactly 0 (no mask tensor shipped);
    all-pad rows are saved from 0/0 by a denominator clamp.
  - Softmax division is deferred past the segment sum; the weighted sum is
    D_t PSUM-accumulated identity-lhsT matmuls.
  - The layer tables are computed shard-locally and AllGather'd.
  - Transfers over the axon tunnel are the wall-clock bottleneck: x ships as
    fp16 in transposed layout, edge indices as uint16, small consts packed
    into one row replicated on device via a rank-1 matmul.  All device_puts
    are issued asynchronously as soon as each host array is ready, and the
    jax/shard_map executable is built once and cached across calls.
"""

import sys

import ml_dtypes
import numpy as np

sys.path.insert(0, "/opt/trn_rl_repo")

F8 = ml_dtypes.float8_e4m3      # what mybir.dt.float8e4 maps to

# ---------------------------------------------------------------- constants
N = 50000
E = 600000
F_IN = 128
HID = 64
NC_CLS = 10
NG = 64
NCORES = 8
NSH_R = N // NCORES          # 6250 real nodes per core
NT = (NSH_R + 127) // 128    # 49 supertiles
NSH = NT * 128               # 6272 padded rank slots per core
TBL_N = NCORES * NSH         # 50176 table rows
POISON = NSH_R               # local rank of the poison row (first pad rank)
F1 = 132                     # L1 table row: 128 feats | attl(2) | pad(2)
F2 = 68                      # L2 table row: 64 feats | attl2(1) | pad(3)
CP = F1 + 128 + F2 + 64      # cpack row: attsc1 | b1 | attsc2 | b2


# ---------------------------------------------------------------- host prep
def _prep_weights(inputs):
    att1 = np.asarray(inputs["att1"], np.float32)
    att2 = np.asarray(inputs["att2"], np.float32)
    Wl1 = np.asarray(inputs["Wl1"], np.float32)
    Wr1 = np.asarray(inputs["Wr1"], np.float32)
    Wl2 = np.asarray(inputs["Wl2"], np.float32)
    Wr2 = np.asarray(inputs["Wr2"], np.float32)
    b1 = np.asarray(inputs["b1"], np.float32)
    b2 = np.asarray(inputs["b2"], np.float32)

    P1 = np.zeros(2 * HID, np.int64)
    k1 = [0, 0]
    Wl1p = np.zeros((F_IN, 2 * HID), np.float32)
    Wr1p = np.zeros((F_IN, 2 * HID), np.float32)
    attsc1 = np.zeros(F1, np.float32)
    for h in (0, 1):
        a = att1[h]
        perm = np.concatenate([np.where(a >= 0)[0], np.where(a < 0)[0]])
        k1[h] = int((a >= 0).sum())
        blk = slice(h * HID, (h + 1) * HID)
        P1[blk] = h * HID + perm
        Wl1p[:, blk] = Wl1[:, blk][:, perm]
        Wr1p[:, blk] = Wr1[:, blk][:, perm]
        attsc1[h * HID:(h + 1) * HID] = 0.4 * np.abs(a[perm])
    wattl1 = 0.6 * np.stack([Wl1[:, h * HID:(h + 1) * HID] @ att1[h]
                             for h in (0, 1)], 1)
    wattr1 = 0.6 * np.stack([Wr1[:, h * HID:(h + 1) * HID] @ att1[h]
                             for h in (0, 1)], 1)
    z2 = np.zeros((F_IN, 2), np.float32)
    Wlp1 = np.concatenate([Wl1p, wattl1, z2], 1).astype(np.float16)
    Wrp1 = np.concatenate([Wr1p, wattr1, z2], 1).astype(np.float16)

    Wl2d = Wl2[P1, :]
    Wr2d = Wr2[P1, :]
    a2 = att2[0]
    P2 = np.concatenate([np.where(a2 >= 0)[0], np.where(a2 < 0)[0]])
    k2 = int((a2 >= 0).sum())
    attsc2 = np.zeros(F2, np.float32)
    attsc2[:HID] = 0.4 * np.abs(a2[P2])
    wattl2 = 0.6 * (Wl2d @ a2)[:, None]
    wattr2 = 0.6 * (Wr2d @ a2)[:, None]
    z3 = np.zeros((2 * HID, 3), np.float32)
    Wlp2 = np.concatenate([Wl2d[:, P2], wattl2, z3], 1).astype(np.float32)
    Wrp2 = np.concatenate([Wr2d[:, P2], wattr2, z3], 1).astype(np.float32)

    cpack = np.concatenate([attsc1, b1[P1], attsc2, b2[P2]]).astype(np.float32)
    return dict(Wlp1=Wlp1, Wrp1=Wrp1, Wlp2=Wlp2, Wrp2=Wrp2, cpack=cpack,
                P2=P2, k1=k1, k2=k2)


def _prep_graph(ei):
    """Degree-sort node partition + supertile degree profile."""
    src = np.concatenate([ei[0].astype(np.int32),
                          np.arange(N, dtype=np.int32)])
    dst = np.concatenate([ei[1].astype(np.int32),
                          np.arange(N, dtype=np.int32)])
    deg = np.bincount(dst, minlength=N).astype(np.int32)
    assert deg.max() <= 128, f"max degree {deg.max()} > 128"
    deg2 = deg.reshape(NCORES, NSH_R)
    order = np.argsort(-deg2, axis=1, kind="stable")
    degs = np.take_along_axis(deg2, order, axis=1)
    degsp = np.zeros((NCORES, NSH), np.int32)
    degsp[:, :NSH_R] = degs
    D = np.maximum(degsp[:, ::128].max(axis=0), 1)
    off = np.concatenate([[0], np.cumsum(D)]).astype(np.int64)
    perm_nodes = order + (np.arange(NCORES, dtype=np.int32) * NSH_R)[:, None]
    rank_of = np.empty(N, np.int32)
    rank_of[perm_nodes.ravel()] = np.tile(
        np.arange(NSH_R, dtype=np.int32), NCORES)
    return src, dst, D, off, perm_nodes, rank_of


XQ_SCALE = 32.0      # u8 linear quant step for x: q = round(x*32)+128


def _quant_x(x):
    """u8-quantize x in natural node order (the device permutation-gathers
    rows per supertile).  floor(y+0.5) == round-half-up, done by the u8 cast."""
    t = x * XQ_SCALE
    t += 128.5
    np.clip(t, 0.0, 255.0, out=t)
    return t.astype(np.uint8)


def _build_perm(perm_nodes):
    """Local node id of each (partition, supertile) slot, u16."""
    op = np.zeros((NCORES, NSH), np.uint16)
    op[:, :NSH_R] = perm_nodes - (np.arange(NCORES) * NSH_R)[:, None]
    return np.ascontiguousarray(
        op.reshape(NCORES, NT, 128).transpose(0, 2, 1)).reshape(-1, NT)


def _build_edges(src, dst, D, off, rank_of, SD):
    gkey = ((dst // NSH_R) * NSH + rank_of[dst]).astype(np.uint16)
    eorder = np.argsort(gkey, kind="stable")   # 2-pass radix on u16
    gs = gkey[eorder].astype(np.int32)
    vals = ((src // NSH_R) * NSH + rank_of[src]).astype(np.uint16)[eorder]
    starts = np.searchsorted(gs, np.arange(TBL_N + 1, dtype=np.int32)
                             ).astype(np.int64)
    slot = np.arange(len(gs), dtype=np.int64) - starts[gs]
    c_e = gs // NSH
    r_e = gs % NSH
    idx_cat = np.full((NCORES * 128, SD), POISON, np.uint16)
    idx_cat[c_e * 128 + (r_e & 127), off[r_e >> 7] + slot] = vals
    return idx_cat


def _build_batch(batch_np, perm_nodes):
    bpad = np.full((NCORES, NSH), -1.0, np.float32)
    bpad[:, :NSH_R] = batch_np[perm_nodes].astype(np.float32)
    return np.ascontiguousarray(
        bpad.reshape(NCORES, NT, 128).transpose(0, 2, 1)
    ).reshape(NCORES * 128, NT)


def prep(inputs):
    """Full host-side restructuring (single-shot path used by the mock)."""
    w = _prep_weights(inputs)
    ei = np.asarray(inputs["edge_index"])
    src, dst, D, off, perm_nodes, rank_of = _prep_graph(ei)
    SD = int(D.sum())
    static = dict(D=[int(d) for d in D], SD=SD, k1=w["k1"], k2=w["k2"])
    x = np.asarray(inputs["x"], np.float32)
    arrs = {
        "xq": _quant_x(x),
        "permv": _build_perm(perm_nodes),
        "wg16": np.tile(np.concatenate([w["Wlp1"], w["Wrp1"]], 1)[None],
                        (NCORES, 1, 1)).reshape(-1, 2 * F1),
        "wg32": np.concatenate([
            np.tile(np.concatenate([w["Wlp2"], w["Wrp2"]], 1)[None],
                    (NCORES, 1, 1)).reshape(-1, 2 * F2),
            _build_batch(np.asarray(inputs["batch"]).astype(np.int32),
                         perm_nodes)], 1),
        "idxu": _build_edges(src, dst, D, off, rank_of, SD),
        "cpack": np.tile(w["cpack"][None], (NCORES, 1)),
    }
    host_ctx = dict(
        batch=np.asarray(inputs["batch"]).astype(np.int32), P2=w["P2"],
        Wlin=np.asarray(inputs["Wlin"], np.float32),
        blin=np.asarray(inputs["blin"], np.float32),
    )
    return static, arrs, host_ctx


def host_epilogue(pooled, host_ctx):
    """pooled: [NG, HID] already summed across cores (device AllReduce)."""
    counts = np.bincount(host_ctx["batch"], minlength=NG).astype(np.float32)
    g = pooled / np.maximum(counts, 1.0)[:, None]
    Wlin_p = host_ctx["Wlin"][host_ctx["P2"], :]
    return (g @ Wlin_p + host_ctx["blin"]).astype(np.float32)


# ---------------------------------------------------------------- numpy mock
def numpy_device_mock(static, arrs, host_ctx):
    """Bit-faithful-ish (fp32 with fp16-rounded inputs) simulation of the
    device kernel.  Used to validate host-side restructuring off-hardware."""
    D, SD = static["D"], static["SD"]
    off = np.concatenate([[0], np.cumsum(D)]).astype(np.int64)
    k1, k2 = static["k1"], static["k2"]
    xq = np.asarray(arrs["xq"]).astype(np.float32)
    x16 = ((xq - 128.0) * np.float32(1.0 / XQ_SCALE)).astype(np.float16)
    permv = arrs["permv"].reshape(NCORES, 128, NT).astype(np.int64)
    xg = np.zeros((NCORES, NSH, F_IN), np.float16)
    for c in range(NCORES):
        loc = np.ascontiguousarray(
            permv[c].T).reshape(-1)               # rank -> local node id
        xg[c] = x16.reshape(NCORES, NSH_R, F_IN)[c][loc]
    xT = np.ascontiguousarray(xg.transpose(0, 2, 1)).astype(np.float32)
    wlp1 = arrs["wg16"][:F_IN, 0:F1].astype(np.float32)
    wrp1 = arrs["wg16"][:F_IN, F1:2 * F1].astype(np.float32)
    wlp2 = arrs["wg32"][:2 * HID, 0:F2]
    wrp2 = arrs["wg32"][:2 * HID, F2:2 * F2]
    idx = arrs["idxu"].reshape(NCORES, 128, SD).astype(np.int64)
    cpk = arrs["cpack"][0]
    attsc1 = cpk[0:F1]
    b1r = cpk[F1:F1 + 128]
    attsc2 = cpk[F1 + 128:F1 + 128 + F2]
    b2r = cpk[F1 + 128 + F2:CP]
    batchv = arrs["wg32"][:, 2 * F2:2 * F2 + NT].reshape(NCORES, 128, NT)

    def edge_layer(tbl, xre, Fw, nheads, kpos, attsc, brow, h_w):
        h_all = np.zeros((NCORES, 128, NT * h_w), np.float32)
        for c in range(NCORES):
            for t in range(NT):
                d = D[t]
                A = tbl[idx[c, :, off[t]:off[t] + d].reshape(-1)].reshape(
                    128, d, Fw)
                xr = xre[c, :, t * Fw:(t + 1) * Fw]
                s = (A + xr[:, None, :]) * attsc[None, None, :]
                e = np.zeros((128, nheads, d), np.float32)
                for h in range(nheads):
                    base = h * HID
                    pos = np.abs(s[:, :, base:base + kpos[h]]).sum(2)
                    neg = np.abs(s[:, :, base + kpos[h]:base + HID]).sum(2)
                    attl = A[:, :, h_w + h] if Fw == F1 else A[:, :, HID + h]
                    attr = xr[:, (128 if Fw == F1 else HID) + h]
                    e[:, h] = (attl + attr[:, None]) + (pos - neg)
                p = np.exp(e)
                den = np.maximum(p.sum(2), 1e-30)
                outw = np.zeros((128, h_w), np.float32)
                for h in range(nheads):
                    outw[:, h * HID:(h + 1) * HID] = (
                        A[:, :, h * HID:(h + 1) * HID]
                        * p[:, h, :, None]).sum(1) / den[:, h:h + 1]
                hh = outw + brow[None, :h_w]
                hh = np.maximum(hh, np.exp(np.minimum(hh, 0.0)) - 1.0)
                h_all[c, :, t * h_w:(t + 1) * h_w] = hh
        return h_all

    tbl1 = np.zeros((TBL_N, F1), np.float32)
    xre1 = np.zeros((NCORES, 128, NT * F1), np.float32)
    for c in range(NCORES):
        for t in range(NT):
            xsl = xT[c][:, t * 128:(t + 1) * 128]
            tbl1[c * NSH + t * 128:c * NSH + (t + 1) * 128] = xsl.T @ wlp1
            xre1[c, :, t * F1:(t + 1) * F1] = xsl.T @ wrp1
    tbl1[np.arange(NCORES) * NSH + POISON, 128:130] = -1e4
    h1 = edge_layer(tbl1, xre1, F1, 2, k1, attsc1, cpk[F1:F1 + 128], 128)

    tbl2 = np.zeros((TBL_N, F2), np.float32)
    xre2 = np.zeros((NCORES, 128, NT * F2), np.float32)
    for c in range(NCORES):
        for t in range(NT):
            h1t = h1[c, :, t * 128:(t + 1) * 128]
            tbl2[c * NSH + t * 128:c * NSH + (t + 1) * 128] = h1t @ wlp2
            xre2[c, :, t * F2:(t + 1) * F2] = h1t @ wrp2
    tbl2[np.arange(NCORES) * NSH + POISON, HID:HID + 1] = -1e4
    h2 = edge_layer(tbl2, xre2, F2, 1, [k2], attsc2, b2r, HID)

    pooled = np.zeros((NCORES, NG, HID), np.float32)
    for c in range(NCORES):
        for t in range(NT):
            onehot = (np.arange(NG, dtype=np.float32)[None, :]
                      == batchv[c, :, t:t + 1]).astype(np.float32)
            pooled[c] += onehot.T @ h2[c, :, t * HID:(t + 1) * HID]
    return host_epilogue(pooled.sum(0), host_ctx)


# ---------------------------------------------------------------- device impl
def build_nc(static):
    import concourse.bass as bass
    import concourse.bacc as bacc
    import concourse.mybir as mybir
    import concourse.tile as tile
    from contextlib import ExitStack

    fp32 = mybir.dt.float32
    fp16 = mybir.dt.float16
    i32 = mybir.dt.int32
    u16 = mybir.dt.uint16
    u8 = mybir.dt.uint8
    AF = mybir.ActivationFunctionType
    OP = mybir.AluOpType

    D, SD = static["D"], static["SD"]
    off = np.concatenate([[0], np.cumsum(D)]).astype(np.int64)
    k1, k2 = static["k1"], static["k2"]

    nc = bacc.Bacc(None, num_devices=NCORES)

    # ---- I/O ----
    xq = nc.dram_tensor("xq", [NSH_R, F_IN], u8, kind="ExternalInput")
    permv = nc.dram_tensor("permv", [128, NT], u16, kind="ExternalInput")
    wg16 = nc.dram_tensor("wg16", [F_IN, 2 * F1], fp16, kind="ExternalInput")
    wg32 = nc.dram_tensor("wg32", [128, 2 * F2 + NT], fp32,
                          kind="ExternalInput")
    idxu = nc.dram_tensor("idxu", [128, SD], u16, kind="ExternalInput")
    cpack = nc.dram_tensor("cpack", [1, CP], fp32, kind="ExternalInput")
    pooled_out = nc.dram_tensor("pooled", [NG, HID], fp32,
                                kind="ExternalOutput")

    # collective buffers (internal DRAM)
    tbl1_sh = nc.dram_tensor("tbl1_sh", [NSH, F1], fp32)
    tbl1 = nc.dram_tensor("tbl1", [TBL_N, F1], fp32, addr_space="Shared")
    tbl2_sh = nc.dram_tensor("tbl2_sh", [NSH, F2], fp32)
    tbl2 = nc.dram_tensor("tbl2", [TBL_N, F2], fp32, addr_space="Shared")
    pool_loc = nc.dram_tensor("pool_loc", [NG, HID], fp32)
    pool_red = nc.dram_tensor("pool_red", [NG, HID], fp32)

    with tile.TileContext(nc) as tc, ExitStack() as ctx:
        cp = ctx.enter_context(tc.tile_pool(name="const", bufs=1))
        wg16_s = cp.tile([F_IN, 2 * F1], fp16)
        nc.sync.dma_start(wg16_s[:], wg16[:, :])
        wlp1_s = wg16_s[:, 0:F1]
        wrp1_s = wg16_s[:, F1:2 * F1]
        wg32_s = cp.tile([128, 2 * F2 + NT], fp32)
        nc.sync.dma_start(wg32_s[:], wg32[:, :])
        wlp2_s = wg32_s[:, 0:F2]
        wrp2_s = wg32_s[:, F2:2 * F2]
        batch_s = wg32_s[:, 2 * F2:2 * F2 + NT]
        cpk_s = cp.tile([1, CP], fp32); nc.sync.dma_start(cpk_s[:], cpack[:, :])
        idxu_s = cp.tile([128, SD], u16); nc.sync.dma_start(idxu_s[:], idxu[:, :])
        idx32_s = cp.tile([128, SD], i32)
        nc.vector.tensor_scalar(idx32_s[:], idxu_s[:], 0, None, op0=OP.add)
        permu_s = cp.tile([128, NT], u16); nc.sync.dma_start(permu_s[:], permv[:, :])
        perm32_s = cp.tile([128, NT], i32)
        nc.vector.tensor_scalar(perm32_s[:], permu_s[:], 0, None, op0=OP.add)

        ones_s = cp.tile([1, 128], fp32); nc.vector.memset(ones_s[:], 1.0)
        pois_s = cp.tile([1, 2], fp32); nc.vector.memset(pois_s[:], -1e4)
        iotaF_i = cp.tile([128, 128], i32)
        nc.gpsimd.iota(iotaF_i[:], [[1, 128]], channel_multiplier=0)
        iotaP_i = cp.tile([128, 1], i32)
        nc.gpsimd.iota(iotaP_i[:], [[1, 1]], channel_multiplier=1)
        iotaF_f = cp.tile([128, 128], fp32)
        nc.vector.tensor_scalar(iotaF_f[:], iotaF_i[:], 0, None, op0=OP.add)
        iotaP_f = cp.tile([128, 1], fp32)
        nc.vector.tensor_scalar(iotaP_f[:], iotaP_i[:], 0, None, op0=OP.add)
        id_s = cp.tile([128, 128], fp32)
        nc.vector.tensor_scalar(id_s[:], iotaF_f[:], iotaP_f[:, 0:1], None,
                                op0=OP.is_equal)
        id16_s = cp.tile([128, 128], fp16)
        nc.vector.tensor_scalar(id16_s[:], id_s[:], 0.0, None, op0=OP.add)
        io64_s = iotaF_f[:, 0:NG]

        consts_s = cp.tile([128, CP], fp32)
        with tc.tile_pool(name="init_ps", bufs=1, space="PSUM") as ip:
            psC = ip.tile([128, CP], fp32)
            nc.tensor.matmul(psC[:], ones_s[:], cpk_s[:], start=True, stop=True)
            nc.scalar.copy(consts_s[:], psC[:])
        attsc1_s = consts_s[:, 0:F1]
        b1_s = consts_s[:, F1:F1 + 128]
        attsc2_s = consts_s[:, F1 + 128:F1 + 128 + F2]
        b2_s = consts_s[:, F1 + 128 + F2:CP]

        big = ctx.enter_context(tc.tile_pool(name="big", bufs=1))
        xre1_s = big.tile([128, NT * F1], fp32)
        h1_s = big.tile([128, NT * 128], fp32)

        # ---------------- phase A: layer-1 tables ----------------
        # x arrives [nodes, feat] u8-quantized in natural node order; per
        # supertile: permutation-gather 128 rows, dequantize to fp16,
        # PE-transpose, then the two table matmuls
        with tc.tile_pool(name="phA", bufs=3) as pa, \
             tc.tile_pool(name="phA_ps", bufs=3, space="PSUM") as pap:
            for t in range(NT):
                xqt = pa.tile([128, F_IN], u8, tag="xqt")
                nc.gpsimd.indirect_dma_start(
                    out=xqt[:],
                    out_offset=None,
                    in_=xq[:, :],
                    in_offset=bass.IndirectOffsetOnAxis(
                        ap=perm32_s[:, t:t + 1], axis=0),
                )
                x16t = pa.tile([128, F_IN], fp16, tag="x16t")
                nc.vector.tensor_scalar(x16t[:], xqt[:], -128.0,
                                        float(1.0 / XQ_SCALE),
                                        op0=OP.add, op1=OP.mult)
                psT = pap.tile([128, 128], fp16, tag="psT")
                nc.tensor.transpose(psT[:], x16t[:], id16_s[:])
                lhs = pa.tile([128, 128], fp16, tag="xTt")
                nc.scalar.copy(lhs[:], psT[:])
                ps = pap.tile([128, F1], fp32, tag="psA")
                nc.tensor.matmul(ps[:], lhs[:], wlp1_s, start=True, stop=True)
                stg = pa.tile([128, F1], fp32, tag="stgA")
                nc.scalar.copy(stg[:], ps[:])
                nc.sync.dma_start(tbl1_sh[t * 128:(t + 1) * 128, :], stg[:])
                ps2 = pap.tile([128, F1], fp32, tag="psA")
                nc.tensor.matmul(ps2[:], lhs[:], wrp1_s, start=True, stop=True)
                nc.scalar.copy(xre1_s[:, t * F1:(t + 1) * F1], ps2[:])
        nc.sync.dma_start(tbl1_sh[POISON:POISON + 1, 128:130], pois_s[0:1, 0:2])

        nc.gpsimd.collective_compute(
            "AllGather", mybir.AluOpType.bypass,
            replica_groups=[list(range(NCORES))],
            ins=[tbl1_sh[:, :]], outs=[tbl1[:, :]],
        )

        # ---------------- edge phase ----------------
        def edge_layer(tblT, xre_s, Fw, nheads, kpos, attsc_s, bt_s, h_out,
                       h_w):
            maxD = max(D)
            with tc.tile_pool(name=f"edg{Fw}", bufs=2) as pe, \
                 tc.tile_pool(name=f"sm{Fw}", bufs=3) as psm, \
                 tc.tile_pool(name=f"ps{Fw}", bufs=2, space="PSUM") as pps:
                for t in range(NT):
                    d = D[t]
                    # idx/out for the indirect DMA must be exact contiguous
                    # tiles (sliced/strided APs crash the DMA engine)
                    idxt = pe.tile([128, d], i32, tag="idxt")
                    nc.vector.tensor_scalar(
                        idxt[:], idx32_s[:, int(off[t]):int(off[t]) + d],
                        0, None, op0=OP.add)
                    A = pe.tile([128, d * Fw], fp32, tag="A")
                    # HW indirect DMA honors ONE offset per partition per call
                    for kk in range(d):
                        nc.gpsimd.indirect_dma_start(
                            out=A[:, kk * Fw:(kk + 1) * Fw],
                            out_offset=None,
                            in_=tblT[:, :],
                            in_offset=bass.IndirectOffsetOnAxis(
                                ap=idxt[:, kk:kk + 1], axis=0),
                        )
                    A3 = A[:].rearrange("p (d f) -> p d f", f=Fw)
                    xr = xre_s[:, t * Fw:(t + 1) * Fw]
                    xrb = xr.rearrange("p (o f) -> p o f", o=1).to_broadcast(
                        [128, d, Fw])
                    s = pe.tile([128, maxD * Fw], fp32, tag="s")
                    s3 = s[:, :d * Fw].rearrange("p (d f) -> p d f", f=Fw)
                    nc.vector.tensor_tensor(s3, A3, xrb, op=OP.add)
                    ascb = attsc_s.rearrange("p (o f) -> p o f",
                                             o=1).to_broadcast([128, d, Fw])
                    nc.vector.tensor_tensor(s3, s3, ascb, op=OP.mult)
                    # e-work tile: [pos_h, neg_h] x heads, then e [h, d]
                    ew = psm.tile([128, 4 * maxD], fp32, tag="ew")
                    for h in range(nheads):
                        base = h * HID
                        nc.vector.tensor_reduce(
                            ew[:, (2 * h) * d:(2 * h) * d + d],
                            s3[:, :, base:base + kpos[h]],
                            axis=mybir.AxisListType.X, op=OP.add,
                            apply_absolute_value=True)
                        nc.vector.tensor_reduce(
                            ew[:, (2 * h + 1) * d:(2 * h + 1) * d + d],
                            s3[:, :, base + kpos[h]:base + HID],
                            axis=mybir.AxisListType.X, op=OP.add,
                            apply_absolute_value=True)
                    # pn = pos - neg  -> [128, h, d]
                    pn = psm.tile([128, 2 * maxD], fp32, tag="pn")
                    ew4 = ew[:, :4 * d].rearrange("p (s d) -> p s d", d=d)
                    pnv = pn[:, :nheads * d].rearrange("p (s d) -> p s d", d=d)
                    nc.vector.tensor_tensor(
                        pnv, ew4[:, 0:2 * nheads:2, :],
                        ew4[:, 1:2 * nheads:2, :], op=OP.subtract)
                    # e = (attl + attr) + pn   (0.6 folded into watt cols)
                    ac = 128 if Fw == F1 else HID
                    attr = xr[:, ac:][:, :nheads]
                    attrb = bass.AP(attr.tensor, attr.offset,
                                    [attr.ap[0], [1, nheads], [0, d]])
                    attlv = bass.AP(A.tensor, A.offset + ac,
                                    [A.ap[0], [1, nheads], [Fw, d]])
                    tmp = psm.tile([128, 2 * maxD], fp32, tag="tmp")
                    tmpv = tmp[:, :nheads * d].rearrange("p (s d) -> p s d", d=d)
                    nc.vector.tensor_tensor(tmpv, attlv, attrb, op=OP.add)
                    ee = psm.tile([128, 2 * maxD], fp32, tag="ee")
                    eev = ee[:, :nheads * d].rearrange("p (s d) -> p s d", d=d)
                    nc.vector.tensor_tensor(eev, tmpv, pnv, op=OP.add)
                    # exp
                    pexp = psm.tile([128, 2 * maxD], fp32, tag="pexp")
                    pexpv = pexp[:, :nheads * d]
                    nc.scalar.activation(pexpv, ee[:, :nheads * d], AF.Exp)
                    pexp3 = pexpv.rearrange("p (s d) -> p s d", d=d)
                    # denom + clamp (all-pad rows sum to exactly 0) + recip
                    den = psm.tile([128, 2], fp32, tag="den")
                    nc.vector.tensor_reduce(den[:, :nheads], pexp3,
                                            axis=mybir.AxisListType.X,
                                            op=OP.add)
                    rd = psm.tile([128, 2], fp32, tag="rd")
                    nc.vector.tensor_scalar(rd[:, :nheads], den[:, :nheads],
                                            1e-30, None, op0=OP.max)
                    nc.vector.reciprocal(rd[:, :nheads], rd[:, :nheads])
                    # W = A * exp  (per head)
                    W = pe.tile([128, maxD * h_w], fp32, tag="W")
                    W3 = W[:, :d * h_w].rearrange("p (d f) -> p d f", f=h_w)
                    for h in range(nheads):
                        eb = bass.AP(pexp.tensor, pexp.offset + h * d,
                                     [pexp.ap[0], [1, d], [0, HID]])
                        nc.vector.tensor_tensor(
                            W3[:, :, h * HID:(h + 1) * HID],
                            A3[:, :, h * HID:(h + 1) * HID], eb, op=OP.mult)
                    # PSUM-accumulated identity matmuls over slots
                    po = pps.tile([128, h_w], fp32, tag="po")
                    for dd in range(d):
                        nc.tensor.matmul(po[:], id_s[:], W3[:, dd, :],
                                         start=(dd == 0), stop=(dd == d - 1))
                    # epilogue: divide by denom (ACT copy*scale), bias, elu
                    hh = psm.tile([128, h_w], fp32, tag="hh")
                    for h in range(nheads):
                        nc.scalar.activation(
                            hh[:, h * HID:(h + 1) * HID],
                            po[:, h * HID:(h + 1) * HID],
                            AF.Copy, bias=0.0, scale=rd[:, h:h + 1])
                    nc.vector.tensor_tensor(hh[:], hh[:], bt_s[:, :h_w],
                                            op=OP.add)
                    # elu: max(x, exp(min(x,0)) - 1)
                    mn = psm.tile([128, h_w], fp32, tag="mn")
                    nc.vector.tensor_scalar(mn[:], hh[:], 0.0, None, op0=OP.min)
                    ex = psm.tile([128, h_w], fp32, tag="ex")
                    nc.scalar.activation(ex[:], mn[:], AF.Exp)
                    nc.vector.scalar_tensor_tensor(
                        h_out[:, t * h_w:(t + 1) * h_w], ex[:], -1.0, hh[:],
                        op0=OP.add, op1=OP.max)

        edge_layer(tbl1, xre1_s, F1, 2, k1, attsc1_s, b1_s, h1_s, 128)

        # ---------------- phase C: layer-2 tables ----------------
        xre2_s = big.tile([128, NT * F2], fp32)
        with tc.tile_pool(name="phC", bufs=3) as pc, \
             tc.tile_pool(name="phC_ps", bufs=3, space="PSUM") as pcp:
            for t in range(NT):
                psT = pcp.tile([128, 128], fp32, tag="psT")
                nc.tensor.transpose(psT[:], h1_s[:, t * 128:(t + 1) * 128],
                                    id_s[:])
                h1T = pc.tile([128, 128], fp32, tag="h1T")
                nc.scalar.copy(h1T[:], psT[:])
                ps = pcp.tile([128, F2], fp32, tag="psC")
                nc.tensor.matmul(ps[:], h1T[:], wlp2_s, start=True, stop=True)
                stg = pc.tile([128, F2], fp32, tag="stgC")
                nc.scalar.copy(stg[:], ps[:])
                nc.sync.dma_start(tbl2_sh[t * 128:(t + 1) * 128, :], stg[:])
                ps2 = pcp.tile([128, F2], fp32, tag="psC")
                nc.tensor.matmul(ps2[:], h1T[:], wrp2_s, start=True, stop=True)
                nc.scalar.copy(xre2_s[:, t * F2:(t + 1) * F2], ps2[:])
        nc.sync.dma_start(tbl2_sh[POISON:POISON + 1, HID:HID + 1],
                          pois_s[0:1, 0:1])

        nc.gpsimd.collective_compute(
            "AllGather", mybir.AluOpType.bypass,
            replica_groups=[list(range(NCORES))],
            ins=[tbl2_sh[:, :]], outs=[tbl2[:, :]],
        )

        # ---------------- phase D: layer-2 edges ----------------
        h2_s = big.tile([128, NT * HID], fp32)
        edge_layer(tbl2, xre2_s, F2, 1, [k2], attsc2_s, b2_s, h2_s, HID)

        # ---------------- phase E: pooling + cross-core reduce ----------------
        with tc.tile_pool(name="phE", bufs=3) as pe_, \
             tc.tile_pool(name="phE_ps", bufs=1, space="PSUM") as pep:
            psP = pep.tile([NG, HID], fp32)
            for t in range(NT):
                oh = pe_.tile([128, NG], fp32, tag="oh")
                nc.vector.tensor_scalar(oh[:], io64_s, batch_s[:, t:t + 1],
                                        None, op0=OP.is_equal)
                nc.tensor.matmul(psP[:], oh[:], h2_s[:, t * HID:(t + 1) * HID],
                                 start=(t == 0), stop=(t == NT - 1))
            stg = pe_.tile([NG, HID], fp32, tag="stgE")
            nc.scalar.copy(stg[:], psP[:])
            nc.sync.dma_start(pool_loc[:, :], stg[:])
            # AllReduce -> every core holds the full pool; the host then
            # fetches a single shard (1 RTT instead of 8)
            nc.gpsimd.collective_compute(
                "AllReduce", mybir.AluOpType.add,
                replica_groups=[list(range(NCORES))],
                ins=[pool_loc[:, :]], outs=[pool_red[:, :]],
            )
            rstg = pe_.tile([NG, HID], fp32, tag="rstg")
            nc.sync.dma_start(rstg[:], pool_red[:, :])
            nc.sync.dma_start(pooled_out[:, :], rstg[:])

    nc.finalize()
    return nc


# ---------------------------------------------------------------- runner
class _Runner:
    """Builds the Bass module + shard_map'd jit executable ONCE; later calls
    reuse it (no retracing).  device_put is async -> callers overlap H2D with
    the rest of host prep."""

    def __init__(self, static):
        import jax
        import concourse.mybir as mybir
        from jax.sharding import Mesh, PartitionSpec, NamedSharding
        from jax.experimental.shard_map import shard_map
        from concourse.bass2jax import (
            _bass_exec_p, partition_id_tensor, install_neuronx_cc_hook)

        install_neuronx_cc_hook()
        self.jax = jax
        nc = build_nc(static)
        self.nc = nc
        pname = nc.partition_id_tensor.name if nc.partition_id_tensor else None
        in_names, out_names, out_avals, zero_shapes = [], [], [], []
        for alloc in nc.m.functions[0].allocations:
            if not isinstance(alloc, mybir.MemoryLocationSet):
                continue
            name = alloc.memorylocations[0].name
            if alloc.kind == "ExternalInput":
                if name != pname:
                    in_names.append(name)
            elif alloc.kind == "ExternalOutput":
                shape = tuple(alloc.tensor_shape)
                dtype = mybir.dt.np(alloc.dtype)
                out_names.append(name)
                out_avals.append(jax.core.ShapedArray(shape, dtype))
                zero_shapes.append((shape, dtype))
        self.dbg_name = None
        if nc.dbg_addr is not None:
            assert not nc.dbg_callbacks
            self.dbg_name = nc.dbg_addr.name
            in_names.append(self.dbg_name)
        n_params = len(in_names)
        all_names = in_names + out_names + ([pname] if pname else [])
        self.in_names = in_names
        self.out_names = out_names

        def _body(*args):
            operands = list(args)
            if pname is not None:
                operands.append(partition_id_tensor())
            return tuple(_bass_exec_p.bind(
                *operands, out_avals=tuple(out_avals),
                in_names=tuple(all_names), out_names=tuple(out_names),
                lowering_input_output_aliases=(),
                sim_require_finite=True, sim_require_nnan=True, nc=nc))

        devices = jax.devices()[:NCORES]
        mesh = Mesh(np.asarray(devices), ("core",))
        self.sharding = NamedSharding(mesh, PartitionSpec("core"))
        nio = n_params + len(out_names)
        # no donation: our kernel writes every output element, so the zero
        # "output seed" buffers can be device-resident constants reused
        # across calls (saves one H2D serialization pass per call)
        self.fn = jax.jit(
            shard_map(_body, mesh=mesh,
                      in_specs=(PartitionSpec("core"),) * nio,
                      out_specs=(PartitionSpec("core"),) * len(out_names),
                      check_rep=False),
            keep_unused=True)
        self.zeros = [self.put(np.zeros((NCORES * s[0],) + tuple(s[1:]), dt))
                      for s, dt in zero_shapes]
        if self.dbg_name is not None:
            self.dbg_zero = self.put(np.zeros((NCORES, 2), np.uint32))

    def put(self, arr):
        return self.jax.device_put(arr, self.sharding)

    def run(self, handles):
        if self.dbg_name is not None and self.dbg_name not in handles:
            handles[self.dbg_name] = self.dbg_zero
        outs = self.fn(*[handles[n] for n in self.in_names], *self.zeros)
        # pooled was AllReduced on device; shard 0 already holds the full sum
        pooled = outs[self.out_names.index("pooled")]
        return np.asarray(pooled.addressable_shards[0].data)


_CACHE = {}


def _get_runner(static):
    key = (tuple(static["D"]), tuple(static["k1"]), static["k2"])
    if key not in _CACHE:
        _CACHE[key] = _Runner(static)
    return _CACHE[key]


def kernel(**inputs) -> np.ndarray:
    w = _prep_weights(inputs)
    ei = np.asarray(inputs["edge_index"])
    src, dst, D, off, perm_nodes, rank_of = _prep_graph(ei)
    SD = int(D.sum())
    static = dict(D=[int(d) for d in D], SD=SD, k1=w["k1"], k2=w["k2"])
    runner = _get_runner(static)

    # x first: device_put is async, the 6.25MB stream while the rest of
    # host prep runs
    x = np.asarray(inputs["x"], np.float32)
    handles = {"xq": runner.put(_quant_x(x))}
    handles["permv"] = runner.put(_build_perm(perm_nodes))
    handles["idxu"] = runner.put(_build_edges(src, dst, D, off, rank_of, SD))
    handles["wg16"] = runner.put(
        np.tile(np.concatenate([w["Wlp1"], w["Wrp1"]], 1)[None],
                (NCORES, 1, 1)).reshape(-1, 2 * F1))
    batch_np = np.asarray(inputs["batch"]).astype(np.int32)
    handles["wg32"] = runner.put(np.concatenate([
        np.tile(np.concatenate([w["Wlp2"], w["Wrp2"]], 1)[None],
                (NCORES, 1, 1)).reshape(-1, 2 * F2),
        _build_batch(batch_np, perm_nodes)], 1))
    handles["cpack"] = runner.put(np.tile(w["cpack"][None], (NCORES, 1)))

    pooled = runner.run(handles)
    host_ctx = dict(batch=batch_np, P2=w["P2"],
                    Wlin=np.asarray(inputs["Wlin"], np.float32),
                    blin=np.asarray(inputs["blin"], np.float32))
    return host_epilogue(pooled, host_ctx)
